# revision 30
# baseline (speedup 1.0000x reference)
"""Trainium2 Bass kernel for nn_BasicTransformerBlock (sparse attention video block).

Strategy (8 NeuronCores, SPMD):
  Phase A (frame-parallel): core i owns frames (2i, 2i+1). LN1 + q/k/v
  projections + sparse-causal attention (keys = frame 0 + previous frame) +
  output projection + residual + LN3 + GEGLU FF + residual.
  Reshard: PE-transpose h2 to token-major, AllToAll so each core ends up
  with all 16 frames of a 128-wide spatial-token slice, PE-transpose back
  into feature-major seq-major layout.
  Phase B (spatial-parallel): LNt + temporal attention over the 16 frames of
  each spatial token (block-diagonal masked 128x128 scores) + out proj +
  residual.

Layouts: activations are feature-major [chan on partitions, tokens on free].
LayerNorm stats use ones-matmul reductions on the TensorEngine (output rows
replicated = free partition-broadcast).  All matmuls are bf16 with fp32 PSUM;
the attn1 residual path stays fp32 (FF residual onward is bf16).

DMA-descriptor discipline (the HW is descriptor-issue-bound on the dynamic
HWDGE rings): weights are pre-arranged on the host so every load lands as
whole-partition-row contiguous descriptors; k/q/v, the padded V blocks, and
the bf16 copy of h1 stay in SBUF instead of round-tripping DRAM with
per-head gather patterns; wf2 is loaded once; wf1 twice (once per token
half).  The AllToAll payload is bf16.  LN affine params are folded into the
projection weights on the host; the attn1 head-merge quirk (channel-major
interleave) is folded into o1w's row permutation on the host.
"""

import math

import numpy as np
import ml_dtypes

import jax
from jax.sharding import Mesh, NamedSharding, PartitionSpec
from jax.experimental.shard_map import shard_map as _shard_map

import concourse.bass as bass
import concourse.bacc as bacc
import concourse.bass2jax as _b2j
import concourse.tile as tile
from concourse import mybir
from concourse.masks import make_identity

BF16 = mybir.dt.bfloat16
F32 = mybir.dt.float32
AF = mybir.ActivationFunctionType
OP = mybir.AluOpType

H, DH, C, F, D = 8, 80, 640, 16, 1024
INNER = 4 * C            # 2560
NI = INNER // 128        # 20
NCORES = 8
KT = C // 128            # 5 feature k-tiles
NQ = 2 * D               # 2048 own tokens / core
NKV = 3 * D              # 3072 kv-context tokens / core
NU = 4 * D               # 4096 union tokens / core
EPS = 1e-5
RG = [list(range(NCORES))]

_BUILD_CACHE = {}


def _build(reps=1, use_cc=True, upto=4):
    key = (reps, use_cc, upto)
    if key in _BUILD_CACHE:
        return _BUILD_CACHE[key]
    nc = bacc.Bacc("TRN2", target_bir_lowering=False, debug=False,
                   num_devices=NCORES)

    def din(name, shape, dt):
        return nc.dram_tensor(name, shape, dt, kind="ExternalInput").ap()

    xub_d = din("xub", [KT, 128, NU], BF16)     # union x, feature-major bf16
    xuq_d = din("xuq", [KT, 128, NQ], F32)      # raw own x (residual), fm f32
    wq_d = din("wq", [128, KT, C], BF16)
    wk_d = din("wk", [128, KT, C], BF16)
    wv_d = din("wv", [128, KT, C], BF16)
    wo1_d = din("wo1", [H, 81, C], BF16)        # per-head o-proj (row 80 = 0)
    wf1a_d = din("wf1a", [NI, 128, KT, 128], BF16)
    wf1g_d = din("wf1g", [NI, 128, KT, 128], BF16)
    wf2_d = din("wf2", [128, NI, KT, 128], BF16)
    wqt_d = din("wqt", [128, KT, C], BF16)
    wkt_d = din("wkt", [128, KT, C], BF16)
    wvt_d = din("wvt", [128, KT, C], BF16)
    wot_d = din("wot", [81, H, KT, 128], BF16)
    bqh_d = din("bqh", [80, H], F32)            # head-major [dh, h]
    bkh_d = din("bkh", [80, H], F32)
    bvbc_d = din("bvbc", [128, C], F32)         # partition-broadcast v bias
    bo1_d = din("bo1", [128, KT], F32)          # [p, m] = b[m*128+p]
    bf1c_d = din("bf1c", [128, 2 * NI], F32)    # cols 0..19 a, 20..39 g
    bf2_d = din("bf2", [128, KT], F32)
    bqth_d = din("bqth", [80, H], F32)
    bkth_d = din("bkth", [80, H], F32)
    bvtbc_d = din("bvtbc", [128, C], F32)
    bot_d = din("bot", [128, KT], F32)
    mask_d = din("mask", [128, 128], BF16)
    out_d = nc.dram_tensor("out", [KT, 128, NQ], F32,
                           kind="ExternalOutput").ap()

    with tile.TileContext(nc) as tc:
        with tc.tile_pool(name="const", bufs=1) as constp, \
             tc.tile_pool(name="dram", bufs=1, space="DRAM") as dramp, \
             tc.tile_pool(name="ps512", bufs=2, space="PSUM") as ps512, \
             tc.tile_pool(name="pspv", bufs=1, space="PSUM") as pspv:

            # ---------------- global constants -------------
            ident = constp.tile([128, 128], F32)
            make_identity(nc, ident)
            identb = constp.tile([128, 128], BF16)
            nc.vector.tensor_copy(identb, ident)
            ones_b = constp.tile([128, 128], BF16)
            nc.vector.memset(ones_b, 1.0)
            epst = constp.tile([128, 1], F32)
            nc.vector.memset(epst, EPS)
            mask_t = constp.tile([128, 128], BF16)
            nc.sync.dma_start(out=mask_t, in_=mask_d[:, :])

            def const_tile(d, shape, tag):
                t = constp.tile(shape, F32, tag=tag, name=tag)
                nc.sync.dma_start(out=t, in_=d[:, :])
                return t

            bqh_t = const_tile(bqh_d, [80, H], "bqh")
            bkh_t = const_tile(bkh_d, [80, H], "bkh")
            bvbc = const_tile(bvbc_d, [128, C], "bvbc")
            bo1_t = const_tile(bo1_d, [128, KT], "bo1")
            bf1c_t = const_tile(bf1c_d, [128, 2 * NI], "bf1c")
            bf2_t = const_tile(bf2_d, [128, KT], "bf2")
            bqth_t = const_tile(bqth_d, [80, H], "bqth")
            bkth_t = const_tile(bkth_d, [80, H], "bkth")
            bvtbc = const_tile(bvtbc_d, [128, C], "bvtbc")
            bot_t = const_tile(bot_d, [128, KT], "bot")

            def chunk_stats(wkp, stripes):
                """stripes: KT bf16 [128, 512] views of one token chunk.
                LayerNorm is token-local, so mean/rstd complete per chunk.
                Returns (M, R) bf16 [128, 512] replicated across partitions."""
                M = wkp.tile([128, 512], BF16, tag="Mch")
                R = wkp.tile([128, 512], BF16, tag="Rch")
                ps_s = ps512.tile([128, 512], F32, tag="ps")
                for kt in range(KT):
                    nc.tensor.matmul(ps_s, ones_b, stripes[kt],
                                     start=(kt == 0), stop=(kt == KT - 1))
                ps_q = ps512.tile([128, 512], F32, tag="ps")
                for kt in range(KT):
                    sq = wkp.tile([128, 512], BF16, tag="sq")
                    nc.vector.tensor_mul(sq, stripes[kt], stripes[kt])
                    nc.tensor.matmul(ps_q, ones_b, sq,
                                     start=(kt == 0), stop=(kt == KT - 1))
                Mf = wkp.tile([128, 512], F32, tag="Mf", bufs=1)
                nc.scalar.activation(out=Mf, in_=ps_s, func=AF.Identity,
                                     scale=1.0 / C)
                nc.vector.tensor_copy(M, Mf)
                msq = wkp.tile([128, 512], F32, tag="msq", bufs=1)
                nc.vector.tensor_mul(msq, Mf, Mf)
                var = wkp.tile([128, 512], F32, tag="var", bufs=1)
                nc.vector.scalar_tensor_tensor(
                    out=var, in0=ps_q, scalar=1.0 / C, in1=msq,
                    op0=OP.mult, op1=OP.subtract)
                sd = wkp.tile([128, 512], F32, tag="sd", bufs=1)
                nc.scalar.activation(out=sd, in_=var, func=AF.Sqrt,
                                     bias=epst)
                with nc.allow_low_precision(reason="rstd in bf16 is fine "
                                            "for standardization"):
                    nc.vector.reciprocal(out=R, in_=sd)
                return M, R

            def emit(it):
                # DRAM staging for the collective, split per frame-half so
                # chunk 0's AllToAll overlaps the FF of the second half
                # (fixed tags -> recycled across reps)
                a2a_i = [dramp.tile([NCORES, 128, C], BF16, tag=f"a2a_i{k}",
                                    name=f"a2a_i{k}") for k in range(2)]
                a2a_o = [dramp.tile([NCORES, 128, C], BF16, tag=f"a2a_o{k}",
                                    name=f"a2a_o{k}") for k in range(2)]

                def launch_cc(k):
                    if use_cc:
                        nc.gpsimd.collective_compute(
                            "AllToAll", OP.bypass, replica_groups=RG,
                            ins=[a2a_i[k][:, :, :]], outs=[a2a_o[k][:, :, :]])
                    else:
                        nc.sync.dma_start(out=a2a_o[k][:, :, :],
                                          in_=a2a_i[k][:, :, :])

                with tc.tile_pool(name="ph1b", bufs=1) as ph1b:
                    # bf16 copy of post-attn1 hidden, phases 2-3 (reserve now)
                    h1b = ph1b.tile([128, KT, NQ], BF16, tag="h1b", name="h1b")

                    with tc.tile_pool(name="pkv", bufs=1) as pkv:
                        # head-major k/q + padded V blocks, phases 1-2
                        khsb = pkv.tile([80, H, NKV], BF16, tag="khsb",
                                        name="khsb")
                        qsb = pkv.tile([80, H, NQ], BF16, tag="qsb",
                                       name="qsb")
                        v3sb = pkv.tile([128, NKV // 128, H, 81], BF16,
                                        tag="v3sb", name="v3sb")
                        nc.vector.memset(v3sb[:, :, :, 80], 1.0)

                        # ============ Phase 1: LN1 + q/k/v projections ======
                        with tc.tile_pool(name="p1", bufs=1) as p1, \
                             tc.tile_pool(name="p1w", bufs=2) as p1w:
                            wq_t = p1.tile([128, KT, C], BF16, tag="wq")
                            nc.sync.dma_start(out=wq_t, in_=wq_d[:, :, :])
                            wk_t = p1.tile([128, KT, C], BF16, tag="wk")
                            nc.sync.dma_start(out=wk_t, in_=wk_d[:, :, :])
                            wv_t = p1.tile([128, KT, C], BF16, tag="wv")
                            nc.sync.dma_start(out=wv_t, in_=wv_d[:, :, :])

                            for ch in range(NU // 512):
                                sl = slice(ch * 512, (ch + 1) * 512)
                                eng = nc.scalar if ch % 2 else nc.sync
                                xs = []
                                for kt in range(KT):
                                    t = p1w.tile([128, 512], BF16,
                                                 tag=f"xs{kt}", name=f"xs{kt}")
                                    eng.dma_start(out=t, in_=xub_d[kt, :, sl])
                                    xs.append(t)
                                M1, R1 = chunk_stats(p1w, xs)
                                xh = []
                                for kt in range(KT):
                                    tmp = p1w.tile([128, 512], F32,
                                                   tag="stdtmp")
                                    nc.vector.tensor_sub(tmp, xs[kt], M1)
                                    t = p1w.tile([128, 512], BF16,
                                                 tag=f"xh{kt}", name=f"xh{kt}")
                                    nc.vector.tensor_mul(t, tmp, R1)
                                    xh.append(t)
                                if ch < NKV // 512:      # kv-range: k, v proj
                                    for h in range(H):
                                        hsl = slice(h * DH, (h + 1) * DH)
                                        ps = ps512.tile([80, 512], F32,
                                                        tag="ps")
                                        for kt in range(KT):
                                            nc.tensor.matmul(
                                                ps, wk_t[:, kt, hsl], xh[kt],
                                                start=(kt == 0),
                                                stop=(kt == KT - 1))
                                        nc.scalar.activation(
                                            out=khsb[:, h, sl], in_=ps,
                                            func=AF.Identity,
                                            bias=bkh_t[:, h:h + 1])
                                    for tw in range(4):
                                        tsl = slice(tw * 128, (tw + 1) * 128)
                                        for hf in range(2):
                                            csl = slice(hf * 320,
                                                        (hf + 1) * 320)
                                            ps = ps512.tile([128, 320], F32,
                                                            tag="ps")
                                            for kt in range(KT):
                                                nc.tensor.matmul(
                                                    ps, xh[kt][:, tsl],
                                                    wv_t[:, kt, csl],
                                                    start=(kt == 0),
                                                    stop=(kt == KT - 1))
                                            nc.vector.tensor_add(
                                                v3sb[:, ch * 4 + tw,
                                                     hf * 4:(hf + 1) * 4,
                                                     0:80],
                                                ps.rearrange(
                                                    "p (h c) -> p h c", c=DH),
                                                bvbc[:, csl].rearrange(
                                                    "p (h c) -> p h c", c=DH))
                                if ch >= (NU - NQ) // 512:   # q-range
                                    qsl = slice(ch * 512 - (NU - NQ),
                                                (ch + 1) * 512 - (NU - NQ))
                                    for h in range(H):
                                        hsl = slice(h * DH, (h + 1) * DH)
                                        ps = ps512.tile([80, 512], F32,
                                                        tag="ps")
                                        for kt in range(KT):
                                            nc.tensor.matmul(
                                                ps, wq_t[:, kt, hsl], xh[kt],
                                                start=(kt == 0),
                                                stop=(kt == KT - 1))
                                        nc.scalar.activation(
                                            out=qsb[:, h, qsl], in_=ps,
                                            func=AF.Identity,
                                            bias=bqh_t[:, h:h + 1])

                        # ============ Phase 2: sparse-causal attention ======
                        if upto < 2:
                            nc.sync.dma_start(out=out_d[0, :, 0:C], in_=bvbc)
                            return
                        with tc.tile_pool(name="p2", bufs=1) as p2, \
                             tc.tile_pool(name="p2w", bufs=3) as p2w, \
                             tc.tile_pool(name="p2d", bufs=8) as p2d, \
                             tc.tile_pool(name="psc2", bufs=2,
                                          space="PSUM") as psc2:
                            wo1_t = []
                            for h in range(H):
                                t = p2.tile([81, C], BF16, tag=f"wo1_{h}",
                                            name=f"wo1_{h}")
                                nc.scalar.dma_start(out=t, in_=wo1_d[h])
                                wo1_t.append(t)
                            for fi in range(2):
                                attD = []
                                for h in range(H):
                                    attP = p2w.tile([81, D], F32, tag="attP",
                                                    bufs=2)
                                    pv = pspv.tile([81, D], F32, tag="pv")
                                    for ktile in range(16):
                                        if ktile < 8:
                                            kc = ktile * 128
                                            tv = ktile
                                        else:
                                            kc = (1 + fi) * 1024 \
                                                + (ktile - 8) * 128
                                            tv = 8 * (1 + fi) + (ktile - 8)
                                        for qh in range(2):
                                            qof = fi * D + qh * 512
                                            sc = psc2.tile([128, 512], F32,
                                                           tag="sc2")
                                            nc.tensor.matmul(
                                                sc,
                                                khsb[:, h, kc:kc + 128],
                                                qsb[:, h, qof:qof + 512],
                                                start=True, stop=True)
                                            P = p2w.tile([128, 512], BF16,
                                                         tag="P")
                                            nc.scalar.activation(
                                                out=P, in_=sc, func=AF.Exp)
                                            nc.tensor.matmul(
                                                pv[:, qh * 512:
                                                   (qh + 1) * 512],
                                                v3sb[:, tv, h, :], P,
                                                start=(ktile == 0),
                                                stop=(ktile == 15))
                                    nc.vector.tensor_copy(attP, pv)
                                    dnm0 = p2w.tile([1, D], F32, tag="dnm0",
                                                    bufs=2)
                                    nc.sync.dma_start(out=dnm0,
                                                      in_=attP[80:81, :])
                                    nc.vector.reciprocal(out=dnm0, in_=dnm0)
                                    attB = p2w.tile([80, D], F32, tag="attB",
                                                    bufs=2)
                                    nc.gpsimd.partition_broadcast(
                                        attB, dnm0[0:1, :], channels=80)
                                    aD = p2d.tile([81, D], BF16, tag="attD",
                                                  name="attD")
                                    nc.vector.memset(aD, 0.0)
                                    nc.vector.tensor_mul(aD[0:80, :],
                                                         attP[0:80, :], attB)
                                    attD.append(aD)
                                for m in range(KT):
                                    xuqs = p2w.tile([128, D], F32, tag="xuqs", bufs=2)
                                    nc.scalar.dma_start(
                                        out=xuqs,
                                        in_=xuq_d[m, :, fi * D:(fi + 1) * D])
                                    for qc in range(2):
                                        qsl = slice(qc * 512, (qc + 1) * 512)
                                        ps = ps512.tile([128, 512], F32,
                                                        tag="ps")
                                        for h in range(H):
                                            nc.tensor.matmul(
                                                ps,
                                                wo1_t[h][:,
                                                         m * 128:(m + 1) * 128],
                                                attD[h][:, qsl],
                                                start=(h == 0),
                                                stop=(h == H - 1))
                                        nc.vector.scalar_tensor_tensor(
                                            out=h1b[:, m,
                                                    fi * D + qc * 512:
                                                    fi * D + (qc + 1) * 512],
                                            in0=ps,
                                            scalar=bo1_t[:, m:m + 1],
                                            in1=xuqs[:, qsl],
                                            op0=OP.add, op1=OP.add)

                    # ============ Phase 3: LN3 + GEGLU FF + transpose =======
                    if upto < 3:
                        nc.sync.dma_start(out=out_d[0, :, 0:C], in_=bvbc)
                        return
                    with tc.tile_pool(name="p3", bufs=1) as p3, \
                         tc.tile_pool(name="p3w", bufs=2) as p3w, \
                         tc.tile_pool(name="p3e", bufs=3) as p3e, \
                         tc.tile_pool(name="p3ff", bufs=2,
                                      space="PSUM") as p3ff, \
                         tc.tile_pool(name="pstr", bufs=1,
                                      space="PSUM") as pstr:
                        wf2sb = p3.tile([128, NI, KT, 128], BF16, tag="wf2sb")
                        nc.sync.dma_start(out=wf2sb, in_=wf2_d[:, :, :, :])
                        xh3 = p3.tile([128, KT, NQ], BF16, tag="xh3")
                        for ch in range(NQ // 512):
                            sl = slice(ch * 512, (ch + 1) * 512)
                            M3, R3 = chunk_stats(
                                p3w, [h1b[:, kt, sl] for kt in range(KT)])
                            for kt in range(KT):
                                tmp = p3w.tile([128, 512], F32, tag="stdtmp3")
                                nc.vector.tensor_sub(tmp, h1b[:, kt, sl], M3)
                                nc.vector.tensor_mul(xh3[:, kt, sl], tmp, R3)
                        h2b = p3.tile([128, KT, NQ], BF16, tag="h2b")
                        for half in range(2):
                            hbase = half * D
                            ffin = p3.tile([128, NI, D], BF16, tag="ffin")
                            for j in range(NI):
                                wa = p3e.tile([128, KT, 128], BF16,
                                              tag="wf1a")
                                nc.sync.dma_start(out=wa, in_=wf1a_d[j])
                                wg = p3e.tile([128, KT, 128], BF16,
                                              tag="wf1g")
                                nc.scalar.dma_start(out=wg, in_=wf1g_d[j])
                                for qc in range(2):
                                    sl = slice(hbase + qc * 512,
                                               hbase + (qc + 1) * 512)
                                    psa = ps512.tile([128, 512], F32,
                                                     tag="ps")
                                    psg = p3ff.tile([128, 512], F32,
                                                    tag="ffg")
                                    for kt in range(KT):
                                        nc.tensor.matmul(
                                            psa, wa[:, kt, :],
                                            xh3[:, kt, sl],
                                            start=(kt == 0),
                                            stop=(kt == KT - 1))
                                    for kt in range(KT):
                                        nc.tensor.matmul(
                                            psg, wg[:, kt, :],
                                            xh3[:, kt, sl],
                                            start=(kt == 0),
                                            stop=(kt == KT - 1))
                                    gg = p3e.tile([128, 512], BF16, tag="gg")
                                    nc.scalar.activation(
                                        out=gg, in_=psg, func=AF.Gelu,
                                        bias=bf1c_t[:, NI + j:NI + j + 1])
                                    nc.vector.scalar_tensor_tensor(
                                        out=ffin[:, j,
                                                 qc * 512:(qc + 1) * 512],
                                        in0=psa,
                                        scalar=bf1c_t[:, j:j + 1], in1=gg,
                                        op0=OP.add, op1=OP.mult)
                            for m in range(KT):
                                for qc in range(2):
                                    sl = slice(qc * 512, (qc + 1) * 512)
                                    asl = slice(hbase + qc * 512,
                                                hbase + (qc + 1) * 512)
                                    ps = ps512.tile([128, 512], F32,
                                                    tag="ps")
                                    for j in range(NI):
                                        nc.tensor.matmul(
                                            ps, wf2sb[:, j, m, :],
                                            ffin[:, j, sl],
                                            start=(j == 0),
                                            stop=(j == NI - 1))
                                    nc.vector.scalar_tensor_tensor(
                                        out=h2b[:, m, asl],
                                        in0=ps, scalar=bf2_t[:, m:m + 1],
                                        in1=h1b[:, m, asl],
                                        op0=OP.add, op1=OP.add)
                            # transpose this half -> token-major, stage, and
                            # launch its AllToAll (chunk 0 overlaps half 1)
                            for j in range(NCORES):
                                tt = half * 8 + j
                                tm = p3e.tile([128, C], BF16, tag="tmrow")
                                for kt in range(KT):
                                    tp = pstr.tile([128, 128], BF16,
                                                   tag="tr")
                                    nc.tensor.transpose(
                                        tp,
                                        h2b[:, kt,
                                            tt * 128:(tt + 1) * 128],
                                        identb)
                                    nc.vector.tensor_copy(
                                        tm[:, kt * 128:(kt + 1) * 128], tp)
                                nc.sync.dma_start(out=a2a_i[half][j, :, :],
                                                  in_=tm)
                            launch_cc(half)

                if upto < 4:
                    nc.sync.dma_start(out=out_d[0, :, 0:C], in_=bvbc)
                    return

                # ============ Phase 4: temporal block ============
                with tc.tile_pool(name="p4", bufs=1) as p4, \
                     tc.tile_pool(name="p4s", bufs=2) as p4s, \
                     tc.tile_pool(name="p4w", bufs=3) as p4w, \
                     tc.tile_pool(name="p4d", bufs=8) as p4d, \
                     tc.tile_pool(name="pstr", bufs=2, space="PSUM") as pstr:
                    # ht is the temporal residual; bf16 is within tolerance
                    ht = []
                    for kt in range(KT):
                        t = p4.tile([128, NQ], BF16, tag=f"ht{kt}",
                                    name=f"ht{kt}")
                        ht.append(t)
                    for fl in range(2):
                        for j in range(NCORES):
                            rt = p4w.tile([128, C], BF16, tag="rtrow", bufs=2)
                            nc.sync.dma_start(out=rt, in_=a2a_o[fl][j, :, :])
                            fr = 2 * j + fl
                            for kt in range(KT):
                                tp = pstr.tile([128, 128], BF16, tag="tr")
                                nc.tensor.transpose(
                                    tp, rt[:, kt * 128:(kt + 1) * 128],
                                    identb)
                                dst = ht[kt].rearrange("p (s f) -> p s f",
                                                       f=F)
                                nc.vector.tensor_copy(dst[:, :, fr], tp)

                    htb = []
                    for kt in range(KT):
                        t = p4.tile([128, NQ], BF16, tag=f"htb{kt}",
                                    name=f"htb{kt}")
                        htb.append(t)
                    for ch in range(NQ // 512):
                        sl = slice(ch * 512, (ch + 1) * 512)
                        Mt, Rt = chunk_stats(
                            p4w, [ht[kt][:, sl] for kt in range(KT)])
                        for kt in range(KT):
                            tmp = p4w.tile([128, 512], F32, tag="stdtmpt")
                            nc.vector.tensor_sub(tmp, ht[kt][:, sl], Mt)
                            nc.vector.tensor_mul(htb[kt][:, sl], tmp, Rt)

                    wvt_t = p4.tile([128, KT, C], BF16, tag="wvt")
                    nc.sync.dma_start(out=wvt_t, in_=wvt_d[:, :, :])
                    wqt_t = p4.tile([128, KT, C], BF16, tag="wqt")
                    nc.scalar.dma_start(out=wqt_t, in_=wqt_d[:, :, :])
                    wkt_t = p4.tile([128, KT, C], BF16, tag="wkt")
                    nc.scalar.dma_start(out=wkt_t, in_=wkt_d[:, :, :])
                    wot_t = p4.tile([81, H, KT, 128], BF16, tag="wot")
                    nc.sync.dma_start(out=wot_t, in_=wot_d[:, :, :, :])

                    vt3sb = p4.tile([128, NQ // 128, H, 81], BF16,
                                    tag="vt3sb")
                    nc.vector.memset(vt3sb[:, :, :, 80], 1.0)
                    for tt in range(NQ // 128):
                        tsl = slice(tt * 128, (tt + 1) * 128)
                        for hf in range(2):
                            csl = slice(hf * 320, (hf + 1) * 320)
                            ps = ps512.tile([128, 320], F32, tag="ps")
                            for kt in range(KT):
                                nc.tensor.matmul(ps, htb[kt][:, tsl],
                                                 wvt_t[:, kt, csl],
                                                 start=(kt == 0),
                                                 stop=(kt == KT - 1))
                            nc.vector.tensor_add(
                                vt3sb[:, tt, hf * 4:(hf + 1) * 4, 0:80],
                                ps.rearrange("p (h c) -> p h c", c=DH),
                                bvtbc[:, csl].rearrange("p (h c) -> p h c",
                                                        c=DH))

                    attDt = []
                    for h in range(H):
                        hsl = slice(h * DH, (h + 1) * DH)
                        qth = p4s.tile([80, NQ], BF16, tag="qth", bufs=2)
                        kth = p4s.tile([80, NQ], BF16, tag="kth", bufs=2)
                        for ch in range(NQ // 512):
                            sl = slice(ch * 512, (ch + 1) * 512)
                            ps = ps512.tile([80, 512], F32, tag="ps")
                            for kt in range(KT):
                                nc.tensor.matmul(ps, wqt_t[:, kt, hsl],
                                                 htb[kt][:, sl],
                                                 start=(kt == 0),
                                                 stop=(kt == KT - 1))
                            nc.scalar.activation(out=qth[:, sl], in_=ps,
                                                 func=AF.Identity,
                                                 bias=bqth_t[:, h:h + 1])
                            ps2 = ps512.tile([80, 512], F32, tag="ps")
                            for kt in range(KT):
                                nc.tensor.matmul(ps2, wkt_t[:, kt, hsl],
                                                 htb[kt][:, sl],
                                                 start=(kt == 0),
                                                 stop=(kt == KT - 1))
                            nc.scalar.activation(out=kth[:, sl], in_=ps2,
                                                 func=AF.Identity,
                                                 bias=bkth_t[:, h:h + 1])
                        attP = p4w.tile([81, NQ], F32, tag="attPt", bufs=1)
                        for tt in range(NQ // 128):
                            tsl = slice(tt * 128, (tt + 1) * 128)
                            ps_s = ps512.tile([128, 128], F32, tag="ps")
                            nc.tensor.matmul(ps_s, kth[:, tsl], qth[:, tsl],
                                             start=True, stop=True)
                            Pe = p4w.tile([128, 128], BF16, tag="Pe")
                            nc.scalar.activation(out=Pe, in_=ps_s,
                                                 func=AF.Exp)
                            Pm = p4w.tile([128, 128], BF16, tag="Pm")
                            nc.vector.tensor_mul(Pm, Pe, mask_t)
                            pv = pspv.tile([81, 128], F32, tag="pvt")
                            nc.tensor.matmul(pv, vt3sb[:, tt, h, :], Pm,
                                             start=True, stop=True)
                            nc.vector.tensor_copy(attP[:, tsl], pv)
                        dnm0 = p4w.tile([1, NQ], F32, tag="dnm0t", bufs=2)
                        nc.sync.dma_start(out=dnm0, in_=attP[80:81, :])
                        nc.vector.reciprocal(out=dnm0, in_=dnm0)
                        attB = p4w.tile([80, NQ], F32, tag="attBt", bufs=1)
                        nc.gpsimd.partition_broadcast(attB, dnm0[0:1, :],
                                                      channels=80)
                        aD = p4d.tile([81, NQ], BF16, tag="attDt",
                                      name="attDt")
                        nc.vector.memset(aD, 0.0)
                        nc.vector.tensor_mul(aD[0:80, :], attP[0:80, :],
                                             attB)
                        attDt.append(aD)

                    for m in range(KT):
                        for ch in range(NQ // 512):
                            sl = slice(ch * 512, (ch + 1) * 512)
                            ps = ps512.tile([128, 512], F32, tag="ps")
                            for h in range(H):
                                nc.tensor.matmul(
                                    ps, wot_t[:, h, m, :], attDt[h][:, sl],
                                    start=(h == 0), stop=(h == H - 1))
                            oe = p4w.tile([128, 512], F32, tag="oe", bufs=2)
                            nc.vector.scalar_tensor_tensor(
                                out=oe, in0=ps,
                                scalar=bot_t[:, m:m + 1],
                                in1=ht[m][:, sl], op0=OP.add, op1=OP.add)
                            nc.sync.dma_start(out=out_d[m, :, sl], in_=oe)

            for it in range(reps):
                emit(it)

    nc.compile()
    _BUILD_CACHE[key] = nc
    return nc


def _prep_inputs(hidden_states, ln1_g, ln1_b, q1w, k1w, v1w, o1w, o1b,
                 ln3_g, ln3_b, ff_w1, ff_b1, ff_w2, ff_b2,
                 lnt_g, lnt_b, qtw, ktw, vtw, otw, otb):
    """Host-side weight folding + per-core input shards."""
    bf = ml_dtypes.bfloat16
    sc = 1.0 / math.sqrt(DH)

    def fold(g, b, w):
        return g[:, None] * w, b @ w

    wq, bq = fold(ln1_g, ln1_b, q1w)
    wq, bq = wq * sc, bq * sc
    wk, bk = fold(ln1_g, ln1_b, k1w)
    wv, bv = fold(ln1_g, ln1_b, v1w)
    # o1w quirk: channel-major interleave -> padded per-head [81, C] with the
    # original row dh*H + h at padded position (h, dh); row 80 is zero
    # (multiplies the softmax-denominator row).
    wo1 = np.zeros((H, 81, C), np.float32)
    idx_dh = np.arange(DH)
    for h in range(H):
        wo1[h, 0:DH, :] = o1w[idx_dh * H + h, :]
    wf1, bf1 = fold(ln3_g, ln3_b, ff_w1)
    bf1 = bf1 + ff_b1
    wqt, bqt = fold(lnt_g, lnt_b, qtw)
    wqt, bqt = wqt * sc, bqt * sc
    wkt, bkt = fold(lnt_g, lnt_b, ktw)
    wvt, bvt = fold(lnt_g, lnt_b, vtw)
    wot = np.zeros((H, 81, C), np.float32)
    for h in range(H):
        wot[h, 0:DH, :] = otw[h * DH + idx_dh, :]

    # 8 sequences per 128-token tile; block-diag of 8 16x16 blocks
    mask = np.kron(np.eye(8, dtype=np.float32), np.ones((F, F), np.float32))

    def c(a, dt=bf):
        return np.ascontiguousarray(np.asarray(a, np.float32).astype(dt))

    wf1 = np.asarray(wf1, np.float32)
    wf1a = wf1[:, :INNER].reshape(KT, 128, NI, 128).transpose(2, 1, 0, 3)
    wf1g = wf1[:, INNER:].reshape(KT, 128, NI, 128).transpose(2, 1, 0, 3)
    wf2h = np.asarray(ff_w2, np.float32).reshape(NI, 128, KT, 128) \
        .transpose(1, 0, 2, 3)
    woth = wot.reshape(H, 81, KT, 128).transpose(1, 0, 2, 3)

    def colmaj(b, ncol):
        return np.ascontiguousarray(
            np.asarray(b, np.float32).reshape(ncol, -1).T)

    def pkc(w):
        return c(np.asarray(w, np.float32).reshape(KT, 128, C)
                 .transpose(1, 0, 2))

    shared = dict(
        wq=pkc(wq), wk=pkc(wk), wv=pkc(wv), wo1=c(wo1),
        wf1a=c(wf1a), wf1g=c(wf1g), wf2=c(wf2h),
        wqt=pkc(wqt), wkt=pkc(wkt), wvt=pkc(wvt), wot=c(woth),
        bqh=colmaj(bq, H), bkh=colmaj(bk, H),
        bvbc=np.ascontiguousarray(
            np.broadcast_to(np.asarray(bv, np.float32), (128, C))),
        bo1=colmaj(o1b, KT), bf1c=colmaj(bf1, 2 * NI), bf2=colmaj(ff_b2, KT),
        bqth=colmaj(bqt, H), bkth=colmaj(bkt, H),
        bvtbc=np.ascontiguousarray(
            np.broadcast_to(np.asarray(bvt, np.float32), (128, C))),
        bot=colmaj(otb, KT),
        mask=c(mask),
    )

    hs = np.asarray(hidden_states, np.float32)   # [BF, D, C]
    in_maps = []
    for i in range(NCORES):
        fa, fb = 2 * i, 2 * i + 1
        fprev = max(2 * i - 1, 0)
        frames = [0, fprev, fa, fb]
        xu = hs[frames].reshape(NU, C).T          # [C, NU] feature-major
        m = dict(shared)
        m["xub"] = np.ascontiguousarray(xu.astype(bf).reshape(KT, 128, NU))
        m["xuq"] = np.ascontiguousarray(
            xu[:, NQ:].astype(np.float32).reshape(KT, 128, NQ))
        in_maps.append(m)
    return in_maps


class _Runner:
    """One shard_map jit per build variant, reused across calls.

    The stock run_bass_kernel_spmd path rebuilds the jit closure on every
    call, so each launch re-traces, re-lowers and reloads the NEFF through
    the axon tunnel (seconds).  Building it once keeps steady-state launch
    cost at one dispatch round trip, and device-resident inputs make the
    in-program reps slope an honest measure of per-iteration HW time.
    """

    def __init__(self, nc):
        self.nc = nc
        _b2j.install_neuronx_cc_hook()
        pname = nc.partition_id_tensor.name if nc.partition_id_tensor else None
        in_names, out_names, out_avals, zero_outs = [], [], [], []
        for alloc in nc.m.functions[0].allocations:
            if not isinstance(alloc, mybir.MemoryLocationSet):
                continue
            name = alloc.memorylocations[0].name
            if alloc.kind == "ExternalInput":
                if name != pname:
                    in_names.append(name)
            elif alloc.kind == "ExternalOutput":
                out_names.append(name)
                shape = tuple(alloc.tensor_shape)
                dtype = mybir.dt.np(alloc.dtype)
                out_avals.append(jax.core.ShapedArray(shape, dtype))
                zero_outs.append(np.zeros(shape, dtype))
        self.in_names = in_names[:]
        self.out_names = out_names
        self.out_avals = out_avals
        self.zero_outs = zero_outs
        n_params = len(in_names)
        bind_names = in_names + out_names + ([pname] if pname else [])

        def _body(*args):
            operands = list(args)
            if pname is not None:
                operands.append(_b2j.partition_id_tensor())
            return tuple(_b2j._bass_exec_p.bind(
                *operands, out_avals=tuple(out_avals),
                in_names=tuple(bind_names), out_names=tuple(out_names),
                lowering_input_output_aliases=(),
                sim_require_finite=True, sim_require_nnan=True, nc=nc))

        devices = jax.devices()[:NCORES]
        assert len(devices) == NCORES
        self.mesh = Mesh(np.asarray(devices), ("core",))
        nin = n_params + len(out_names)
        self.sharding = NamedSharding(self.mesh, PartitionSpec("core"))
        self.jit = jax.jit(
            _shard_map(_body, mesh=self.mesh,
                       in_specs=(PartitionSpec("core"),) * nin,
                       out_specs=(PartitionSpec("core"),) * len(out_names),
                       check_rep=False),
            keep_unused=True)

    def _concat(self, in_maps):
        cats = [np.concatenate([np.asarray(m[nm]) for m in in_maps], axis=0)
                for nm in self.in_names]
        cats += [np.zeros((NCORES * z.shape[0], *z.shape[1:]), z.dtype)
                 for z in self.zero_outs]
        return cats

    def put(self, in_maps):
        dev = jax.device_put(self._concat(in_maps),
                             [self.sharding] * (len(self.in_names)
                                                + len(self.zero_outs)))
        jax.block_until_ready(dev)
        return dev

    def exec(self, args):
        out = self.jit(*args)
        jax.block_until_ready(out)
        return out

    def run_host(self, in_maps):
        out_arrs = self.exec(self._concat(in_maps))
        res = []
        for c in range(NCORES):
            res.append({nm: np.asarray(out_arrs[i]).reshape(
                NCORES, *self.out_avals[i].shape)[c]
                for i, nm in enumerate(self.out_names)})
        return res


_RUNNER_CACHE = {}


def _get_runner(reps=1, use_cc=True, upto=4):
    key = (reps, use_cc, upto)
    if key not in _RUNNER_CACHE:
        _RUNNER_CACHE[key] = _Runner(_build(reps=reps, use_cc=use_cc, upto=upto))
    return _RUNNER_CACHE[key]


def kernel(**inputs):
    video_length = int(np.asarray(inputs.pop("video_length")))
    assert video_length == F, f"kernel hardcodes F={F}, got {video_length}"
    in_maps = _prep_inputs(**{k: np.asarray(v) for k, v in inputs.items()})
    for attempt in range(3):
        try:
            results = _get_runner(reps=1).run_host(in_maps)
            break
        except Exception:
            # transient NRT exec-unit failures have been observed to clear on
            # the next launch; rebuild the jit and retry
            if attempt == 2:
                raise
            _RUNNER_CACHE.clear()
            jax.clear_caches()
    out = np.empty((F, D, C), np.float32)
    for i in range(NCORES):
        r = results[i]["out"].reshape(C, D // NCORES, F)   # [c, s, f]
        out[:, i * (D // NCORES):(i + 1) * (D // NCORES), :] = r.transpose(2, 1, 0)
    return out


# revision 32
# speedup vs baseline: 1.0455x; 1.0455x over previous
"""Trainium2 Bass kernel for nn_BasicTransformerBlock (sparse attention video block).

Strategy (8 NeuronCores, SPMD):
  Phase A (frame-parallel): core i owns frames (2i, 2i+1). LN1 + q/k/v
  projections + sparse-causal attention (keys = frame 0 + previous frame) +
  output projection + residual + LN3 + GEGLU FF + residual.
  Reshard: PE-transpose h2 to token-major, AllToAll so each core ends up
  with all 16 frames of a 128-wide spatial-token slice, PE-transpose back
  into feature-major seq-major layout.
  Phase B (spatial-parallel): LNt + temporal attention over the 16 frames of
  each spatial token (block-diagonal masked 128x128 scores) + out proj +
  residual.

Layouts: activations are feature-major [chan on partitions, tokens on free].
LayerNorm stats use ones-matmul reductions on the TensorEngine (output rows
replicated = free partition-broadcast).  All matmuls are bf16 with fp32 PSUM;
the attn1 residual path stays fp32 (FF residual onward is bf16).

DMA-descriptor discipline (the HW is descriptor-issue-bound on the dynamic
HWDGE rings): weights are pre-arranged on the host so every load lands as
whole-partition-row contiguous descriptors; k/q/v, the padded V blocks, and
the bf16 copy of h1 stay in SBUF instead of round-tripping DRAM with
per-head gather patterns; wf2 is loaded once; wf1 twice (once per token
half).  The AllToAll payload is bf16.  LN affine params are folded into the
projection weights on the host; the attn1 head-merge quirk (channel-major
interleave) is folded into o1w's row permutation on the host.
"""

import math

import numpy as np
import ml_dtypes

import jax
from jax.sharding import Mesh, NamedSharding, PartitionSpec
from jax.experimental.shard_map import shard_map as _shard_map

import concourse.bass as bass
import concourse.bacc as bacc
import concourse.bass2jax as _b2j
import concourse.tile as tile
from concourse import mybir
from concourse.masks import make_identity

BF16 = mybir.dt.bfloat16
F32 = mybir.dt.float32
AF = mybir.ActivationFunctionType
OP = mybir.AluOpType

H, DH, C, F, D = 8, 80, 640, 16, 1024
INNER = 4 * C            # 2560
NI = INNER // 128        # 20
NCORES = 8
KT = C // 128            # 5 feature k-tiles
NQ = 2 * D               # 2048 own tokens / core
NKV = 3 * D              # 3072 kv-context tokens / core
NU = 4 * D               # 4096 union tokens / core
EPS = 1e-5
RG = [list(range(NCORES))]

_BUILD_CACHE = {}


def _build(reps=1, use_cc=True, upto=4):
    key = (reps, use_cc, upto)
    if key in _BUILD_CACHE:
        return _BUILD_CACHE[key]
    nc = bacc.Bacc("TRN2", target_bir_lowering=False, debug=False,
                   num_devices=NCORES)

    def din(name, shape, dt):
        return nc.dram_tensor(name, shape, dt, kind="ExternalInput").ap()

    xub_d = din("xub", [KT, 128, NU], BF16)     # union x, feature-major bf16
    xuq_d = din("xuq", [KT, 128, NQ], F32)      # raw own x (residual), fm f32
    wq_d = din("wq", [128, KT, C], BF16)
    wk_d = din("wk", [128, KT, C], BF16)
    wv_d = din("wv", [128, KT, C], BF16)
    wo1_d = din("wo1", [H, 81, C], BF16)        # per-head o-proj (row 80 = 0)
    wf1a_d = din("wf1a", [NI, 128, KT, 128], BF16)
    wf1g_d = din("wf1g", [NI, 128, KT, 128], BF16)
    wf2_d = din("wf2", [128, NI, KT, 128], BF16)
    wqt_d = din("wqt", [128, KT, C], BF16)
    wkt_d = din("wkt", [128, KT, C], BF16)
    wvt_d = din("wvt", [128, KT, C], BF16)
    wot_d = din("wot", [81, H, KT, 128], BF16)
    bqh_d = din("bqh", [80, H], F32)            # head-major [dh, h]
    bkh_d = din("bkh", [80, H], F32)
    bvbc_d = din("bvbc", [128, C], F32)         # partition-broadcast v bias
    bo1_d = din("bo1", [128, KT], F32)          # [p, m] = b[m*128+p]
    bf1c_d = din("bf1c", [128, 2 * NI], F32)    # cols 0..19 a, 20..39 g
    bf2_d = din("bf2", [128, KT], F32)
    bqth_d = din("bqth", [80, H], F32)
    bkth_d = din("bkth", [80, H], F32)
    bvtbc_d = din("bvtbc", [128, C], F32)
    bot_d = din("bot", [128, KT], F32)
    mask_d = din("mask", [128, 128], BF16)
    out_d = nc.dram_tensor("out", [KT, 128, NQ], F32,
                           kind="ExternalOutput").ap()

    with tile.TileContext(nc) as tc:
        with tc.tile_pool(name="const", bufs=1) as constp, \
             tc.tile_pool(name="dram", bufs=1, space="DRAM") as dramp, \
             tc.tile_pool(name="ps512", bufs=2, space="PSUM") as ps512, \
             tc.tile_pool(name="pspv", bufs=1, space="PSUM") as pspv:

            # ---------------- global constants -------------
            ident = constp.tile([128, 128], F32)
            make_identity(nc, ident)
            identb = constp.tile([128, 128], BF16)
            nc.vector.tensor_copy(identb, ident)
            ones_b = constp.tile([128, 128], BF16)
            nc.vector.memset(ones_b, 1.0)
            epst = constp.tile([128, 1], F32)
            nc.vector.memset(epst, EPS)
            mask_t = constp.tile([128, 128], BF16)
            nc.sync.dma_start(out=mask_t, in_=mask_d[:, :])

            def const_tile(d, shape, tag):
                t = constp.tile(shape, F32, tag=tag, name=tag)
                nc.sync.dma_start(out=t, in_=d[:, :])
                return t

            bqh_t = const_tile(bqh_d, [80, H], "bqh")
            bkh_t = const_tile(bkh_d, [80, H], "bkh")
            bvbc = const_tile(bvbc_d, [128, C], "bvbc")
            bo1_t = const_tile(bo1_d, [128, KT], "bo1")
            bf1c_t = const_tile(bf1c_d, [128, 2 * NI], "bf1c")
            bf2_t = const_tile(bf2_d, [128, KT], "bf2")
            bqth_t = const_tile(bqth_d, [80, H], "bqth")
            bkth_t = const_tile(bkth_d, [80, H], "bkth")
            bvtbc = const_tile(bvtbc_d, [128, C], "bvtbc")
            bot_t = const_tile(bot_d, [128, KT], "bot")

            def chunk_stats(wkp, stripes):
                """stripes: KT bf16 [128, 512] views of one token chunk.
                LayerNorm is token-local, so mean/rstd complete per chunk.
                Returns (M, R) bf16 [128, 512] replicated across partitions."""
                M = wkp.tile([128, 512], BF16, tag="Mch")
                R = wkp.tile([128, 512], BF16, tag="Rch")
                ps_s = ps512.tile([128, 512], F32, tag="ps")
                for kt in range(KT):
                    nc.tensor.matmul(ps_s, ones_b, stripes[kt],
                                     start=(kt == 0), stop=(kt == KT - 1))
                ps_q = ps512.tile([128, 512], F32, tag="ps")
                for kt in range(KT):
                    sq = wkp.tile([128, 512], BF16, tag="sq")
                    nc.vector.tensor_mul(sq, stripes[kt], stripes[kt])
                    nc.tensor.matmul(ps_q, ones_b, sq,
                                     start=(kt == 0), stop=(kt == KT - 1))
                Mf = wkp.tile([128, 512], F32, tag="Mf", bufs=1)
                nc.scalar.activation(out=Mf, in_=ps_s, func=AF.Identity,
                                     scale=1.0 / C)
                nc.vector.tensor_copy(M, Mf)
                msq = wkp.tile([128, 512], F32, tag="msq", bufs=1)
                nc.vector.tensor_mul(msq, Mf, Mf)
                var = wkp.tile([128, 512], F32, tag="var", bufs=1)
                nc.vector.scalar_tensor_tensor(
                    out=var, in0=ps_q, scalar=1.0 / C, in1=msq,
                    op0=OP.mult, op1=OP.subtract)
                sd = wkp.tile([128, 512], F32, tag="sd", bufs=1)
                nc.scalar.activation(out=sd, in_=var, func=AF.Sqrt,
                                     bias=epst)
                with nc.allow_low_precision(reason="rstd in bf16 is fine "
                                            "for standardization"):
                    nc.vector.reciprocal(out=R, in_=sd)
                return M, R

            def emit(it):
                # DRAM staging for the collective, split per frame-half so
                # chunk 0's AllToAll overlaps the FF of the second half
                # (fixed tags -> recycled across reps)
                a2a_i = [dramp.tile([NCORES, 128, C], BF16, tag=f"a2a_i{k}",
                                    name=f"a2a_i{k}") for k in range(2)]
                a2a_o = [dramp.tile([NCORES, 128, C], BF16, tag=f"a2a_o{k}",
                                    name=f"a2a_o{k}") for k in range(2)]

                def launch_cc(k):
                    if use_cc:
                        nc.gpsimd.collective_compute(
                            "AllToAll", OP.bypass, replica_groups=RG,
                            ins=[a2a_i[k][:, :, :]], outs=[a2a_o[k][:, :, :]])
                    else:
                        nc.sync.dma_start(out=a2a_o[k][:, :, :],
                                          in_=a2a_i[k][:, :, :])

                with tc.tile_pool(name="ph1b", bufs=1) as ph1b:
                    # bf16 copy of post-attn1 hidden, phases 2-3 (reserve now)
                    h1b = ph1b.tile([128, KT, NQ], BF16, tag="h1b", name="h1b")

                    with tc.tile_pool(name="pkv", bufs=1) as pkv:
                        # head-major k/q + padded V blocks, phases 1-2
                        khsb = pkv.tile([80, H, NKV], BF16, tag="khsb",
                                        name="khsb")
                        qsb = pkv.tile([80, H, NQ], BF16, tag="qsb",
                                       name="qsb")
                        v3sb = pkv.tile([128, NKV // 128, H, 81], BF16,
                                        tag="v3sb", name="v3sb")
                        nc.vector.memset(v3sb[:, :, :, 80], 1.0)

                        # ============ Phase 1: LN1 + q/k/v projections ======
                        with tc.tile_pool(name="p1", bufs=1) as p1, \
                             tc.tile_pool(name="p1w", bufs=2) as p1w:
                            wq_t = p1.tile([128, KT, C], BF16, tag="wq")
                            nc.sync.dma_start(out=wq_t, in_=wq_d[:, :, :])
                            wk_t = p1.tile([128, KT, C], BF16, tag="wk")
                            nc.sync.dma_start(out=wk_t, in_=wk_d[:, :, :])
                            wv_t = p1.tile([128, KT, C], BF16, tag="wv")
                            nc.sync.dma_start(out=wv_t, in_=wv_d[:, :, :])

                            for ch in range(NU // 512):
                                sl = slice(ch * 512, (ch + 1) * 512)
                                eng = nc.scalar if ch % 2 else nc.sync
                                xs = []
                                for kt in range(KT):
                                    t = p1w.tile([128, 512], BF16,
                                                 tag=f"xs{kt}", name=f"xs{kt}")
                                    eng.dma_start(out=t, in_=xub_d[kt, :, sl])
                                    xs.append(t)
                                M1, R1 = chunk_stats(p1w, xs)
                                xh = []
                                for kt in range(KT):
                                    tmp = p1w.tile([128, 512], F32,
                                                   tag="stdtmp")
                                    nc.vector.tensor_sub(tmp, xs[kt], M1)
                                    t = p1w.tile([128, 512], BF16,
                                                 tag=f"xh{kt}", name=f"xh{kt}")
                                    nc.vector.tensor_mul(t, tmp, R1)
                                    xh.append(t)
                                if ch < NKV // 512:      # kv-range: k, v proj
                                    for h in range(H):
                                        hsl = slice(h * DH, (h + 1) * DH)
                                        ps = ps512.tile([80, 512], F32,
                                                        tag="ps")
                                        for kt in range(KT):
                                            nc.tensor.matmul(
                                                ps, wk_t[:, kt, hsl], xh[kt],
                                                start=(kt == 0),
                                                stop=(kt == KT - 1))
                                        nc.scalar.activation(
                                            out=khsb[:, h, sl], in_=ps,
                                            func=AF.Identity,
                                            bias=bkh_t[:, h:h + 1])
                                    for tw in range(4):
                                        tsl = slice(tw * 128, (tw + 1) * 128)
                                        for hf in range(2):
                                            csl = slice(hf * 320,
                                                        (hf + 1) * 320)
                                            ps = ps512.tile([128, 320], F32,
                                                            tag="ps")
                                            for kt in range(KT):
                                                nc.tensor.matmul(
                                                    ps, xh[kt][:, tsl],
                                                    wv_t[:, kt, csl],
                                                    start=(kt == 0),
                                                    stop=(kt == KT - 1))
                                            nc.vector.tensor_add(
                                                v3sb[:, ch * 4 + tw,
                                                     hf * 4:(hf + 1) * 4,
                                                     0:80],
                                                ps.rearrange(
                                                    "p (h c) -> p h c", c=DH),
                                                bvbc[:, csl].rearrange(
                                                    "p (h c) -> p h c", c=DH))
                                if ch >= (NU - NQ) // 512:   # q-range
                                    qsl = slice(ch * 512 - (NU - NQ),
                                                (ch + 1) * 512 - (NU - NQ))
                                    for h in range(H):
                                        hsl = slice(h * DH, (h + 1) * DH)
                                        ps = ps512.tile([80, 512], F32,
                                                        tag="ps")
                                        for kt in range(KT):
                                            nc.tensor.matmul(
                                                ps, wq_t[:, kt, hsl], xh[kt],
                                                start=(kt == 0),
                                                stop=(kt == KT - 1))
                                        nc.scalar.activation(
                                            out=qsb[:, h, qsl], in_=ps,
                                            func=AF.Identity,
                                            bias=bqh_t[:, h:h + 1])

                        # ============ Phase 2: sparse-causal attention ======
                        if upto < 2:
                            nc.sync.dma_start(out=out_d[0, :, 0:C], in_=bvbc)
                            return
                        with tc.tile_pool(name="p2", bufs=1) as p2, \
                             tc.tile_pool(name="p2w", bufs=3) as p2w, \
                             tc.tile_pool(name="p2d", bufs=8) as p2d, \
                             tc.tile_pool(name="psc2", bufs=2,
                                          space="PSUM") as psc2:
                            wo1_t = []
                            for h in range(H):
                                t = p2.tile([81, C], BF16, tag=f"wo1_{h}",
                                            name=f"wo1_{h}")
                                nc.scalar.dma_start(out=t, in_=wo1_d[h])
                                wo1_t.append(t)
                            for fi in range(2):
                                attD = []
                                for h in range(H):
                                    attP = p2w.tile([81, D], F32, tag="attP",
                                                    bufs=2)
                                    pv = pspv.tile([81, D], F32, tag="pv")
                                    for ktile in range(16):
                                        if ktile < 8:
                                            kc = ktile * 128
                                            tv = ktile
                                        else:
                                            kc = (1 + fi) * 1024 \
                                                + (ktile - 8) * 128
                                            tv = 8 * (1 + fi) + (ktile - 8)
                                        for qh in range(2):
                                            qof = fi * D + qh * 512
                                            sc = psc2.tile([128, 512], F32,
                                                           tag="sc2")
                                            nc.tensor.matmul(
                                                sc,
                                                khsb[:, h, kc:kc + 128],
                                                qsb[:, h, qof:qof + 512],
                                                start=True, stop=True)
                                            P = p2w.tile([128, 512], BF16,
                                                         tag="P")
                                            nc.scalar.activation(
                                                out=P, in_=sc, func=AF.Exp)
                                            nc.tensor.matmul(
                                                pv[:, qh * 512:
                                                   (qh + 1) * 512],
                                                v3sb[:, tv, h, :], P,
                                                start=(ktile == 0),
                                                stop=(ktile == 15))
                                    nc.vector.tensor_copy(attP, pv)
                                    dnm0 = p2w.tile([1, D], F32, tag="dnm0",
                                                    bufs=2)
                                    nc.sync.dma_start(out=dnm0,
                                                      in_=attP[80:81, :])
                                    nc.vector.reciprocal(out=dnm0, in_=dnm0)
                                    attB = p2w.tile([80, D], F32, tag="attB",
                                                    bufs=2)
                                    nc.gpsimd.partition_broadcast(
                                        attB, dnm0[0:1, :], channels=80)
                                    aD = p2d.tile([81, D], BF16, tag="attD",
                                                  name="attD")
                                    nc.vector.memset(aD, 0.0)
                                    nc.vector.tensor_mul(aD[0:80, :],
                                                         attP[0:80, :], attB)
                                    attD.append(aD)
                                for m in range(KT):
                                    xuqs = p2w.tile([128, D], F32, tag="xuqs", bufs=2)
                                    nc.scalar.dma_start(
                                        out=xuqs,
                                        in_=xuq_d[m, :, fi * D:(fi + 1) * D])
                                    for qc in range(2):
                                        qsl = slice(qc * 512, (qc + 1) * 512)
                                        ps = ps512.tile([128, 512], F32,
                                                        tag="ps")
                                        for h in range(H):
                                            nc.tensor.matmul(
                                                ps,
                                                wo1_t[h][:,
                                                         m * 128:(m + 1) * 128],
                                                attD[h][:, qsl],
                                                start=(h == 0),
                                                stop=(h == H - 1))
                                        nc.vector.scalar_tensor_tensor(
                                            out=h1b[:, m,
                                                    fi * D + qc * 512:
                                                    fi * D + (qc + 1) * 512],
                                            in0=ps,
                                            scalar=bo1_t[:, m:m + 1],
                                            in1=xuqs[:, qsl],
                                            op0=OP.add, op1=OP.add)

                    # ============ Phase 3: LN3 + GEGLU FF + transpose =======
                    if upto < 3:
                        nc.sync.dma_start(out=out_d[0, :, 0:C], in_=bvbc)
                        return
                    with tc.tile_pool(name="p3", bufs=1) as p3, \
                         tc.tile_pool(name="p3w", bufs=2) as p3w, \
                         tc.tile_pool(name="p3e", bufs=3) as p3e, \
                         tc.tile_pool(name="p3ff", bufs=2,
                                      space="PSUM") as p3ff, \
                         tc.tile_pool(name="pstr", bufs=1,
                                      space="PSUM") as pstr:
                        wf2sb = p3.tile([128, NI, KT, 128], BF16, tag="wf2sb")
                        nc.scalar.dma_start(out=wf2sb, in_=wf2_d[:, :, :, :])
                        xh3 = p3.tile([128, KT, NQ], BF16, tag="xh3")
                        for ch in range(NQ // 512):
                            sl = slice(ch * 512, (ch + 1) * 512)
                            M3, R3 = chunk_stats(
                                p3w, [h1b[:, kt, sl] for kt in range(KT)])
                            for kt in range(KT):
                                tmp = p3w.tile([128, 512], F32, tag="stdtmp3")
                                nc.vector.tensor_sub(tmp, h1b[:, kt, sl], M3)
                                nc.vector.tensor_mul(xh3[:, kt, sl], tmp, R3)
                        h2b = p3.tile([128, KT, NQ], BF16, tag="h2b")
                        for half in range(2):
                            hbase = half * D
                            ffin = p3.tile([128, NI, D], BF16, tag="ffin")
                            for j in range(NI):
                                wa = p3e.tile([128, KT, 128], BF16,
                                              tag="wf1a")
                                nc.sync.dma_start(out=wa, in_=wf1a_d[j])
                                wg = p3e.tile([128, KT, 128], BF16,
                                              tag="wf1g")
                                nc.scalar.dma_start(out=wg, in_=wf1g_d[j])
                                for qc in range(2):
                                    sl = slice(hbase + qc * 512,
                                               hbase + (qc + 1) * 512)
                                    psa = ps512.tile([128, 512], F32,
                                                     tag="ps")
                                    psg = p3ff.tile([128, 512], F32,
                                                    tag="ffg")
                                    for kt in range(KT):
                                        nc.tensor.matmul(
                                            psa, wa[:, kt, :],
                                            xh3[:, kt, sl],
                                            start=(kt == 0),
                                            stop=(kt == KT - 1))
                                    for kt in range(KT):
                                        nc.tensor.matmul(
                                            psg, wg[:, kt, :],
                                            xh3[:, kt, sl],
                                            start=(kt == 0),
                                            stop=(kt == KT - 1))
                                    gg = p3e.tile([128, 512], BF16, tag="gg")
                                    nc.scalar.activation(
                                        out=gg, in_=psg, func=AF.Gelu,
                                        bias=bf1c_t[:, NI + j:NI + j + 1])
                                    nc.vector.scalar_tensor_tensor(
                                        out=ffin[:, j,
                                                 qc * 512:(qc + 1) * 512],
                                        in0=psa,
                                        scalar=bf1c_t[:, j:j + 1], in1=gg,
                                        op0=OP.add, op1=OP.mult)
                            for m in range(KT):
                                for qc in range(2):
                                    sl = slice(qc * 512, (qc + 1) * 512)
                                    asl = slice(hbase + qc * 512,
                                                hbase + (qc + 1) * 512)
                                    ps = ps512.tile([128, 512], F32,
                                                    tag="ps")
                                    for j in range(NI):
                                        nc.tensor.matmul(
                                            ps, wf2sb[:, j, m, :],
                                            ffin[:, j, sl],
                                            start=(j == 0),
                                            stop=(j == NI - 1))
                                    nc.vector.scalar_tensor_tensor(
                                        out=h2b[:, m, asl],
                                        in0=ps, scalar=bf2_t[:, m:m + 1],
                                        in1=h1b[:, m, asl],
                                        op0=OP.add, op1=OP.add)
                            # transpose this half -> token-major, stage, and
                            # launch its AllToAll (chunk 0 overlaps half 1)
                            for j in range(NCORES):
                                tt = half * 8 + j
                                tm = p3e.tile([128, C], BF16, tag="tmrow")
                                for kt in range(KT):
                                    tp = pstr.tile([128, 128], BF16,
                                                   tag="tr")
                                    nc.tensor.transpose(
                                        tp,
                                        h2b[:, kt,
                                            tt * 128:(tt + 1) * 128],
                                        identb)
                                    nc.vector.tensor_copy(
                                        tm[:, kt * 128:(kt + 1) * 128], tp)
                                nc.sync.dma_start(out=a2a_i[half][j, :, :],
                                                  in_=tm)
                            launch_cc(half)

                if upto < 4:
                    nc.sync.dma_start(out=out_d[0, :, 0:C], in_=bvbc)
                    return

                # ============ Phase 4: temporal block ============
                with tc.tile_pool(name="p4", bufs=1) as p4, \
                     tc.tile_pool(name="p4s", bufs=2) as p4s, \
                     tc.tile_pool(name="p4w", bufs=3) as p4w, \
                     tc.tile_pool(name="p4d", bufs=8) as p4d, \
                     tc.tile_pool(name="pstr", bufs=2, space="PSUM") as pstr:
                    # ht is the temporal residual; bf16 is within tolerance
                    ht = []
                    for kt in range(KT):
                        t = p4.tile([128, NQ], BF16, tag=f"ht{kt}",
                                    name=f"ht{kt}")
                        ht.append(t)
                    for fl in range(2):
                        for j in range(NCORES):
                            rt = p4w.tile([128, C], BF16, tag="rtrow", bufs=2)
                            nc.sync.dma_start(out=rt, in_=a2a_o[fl][j, :, :])
                            fr = 2 * j + fl
                            for kt in range(KT):
                                tp = pstr.tile([128, 128], BF16, tag="tr")
                                nc.tensor.transpose(
                                    tp, rt[:, kt * 128:(kt + 1) * 128],
                                    identb)
                                dst = ht[kt].rearrange("p (s f) -> p s f",
                                                       f=F)
                                nc.vector.tensor_copy(dst[:, :, fr], tp)

                    htb = []
                    for kt in range(KT):
                        t = p4.tile([128, NQ], BF16, tag=f"htb{kt}",
                                    name=f"htb{kt}")
                        htb.append(t)
                    for ch in range(NQ // 512):
                        sl = slice(ch * 512, (ch + 1) * 512)
                        Mt, Rt = chunk_stats(
                            p4w, [ht[kt][:, sl] for kt in range(KT)])
                        for kt in range(KT):
                            tmp = p4w.tile([128, 512], F32, tag="stdtmpt")
                            nc.vector.tensor_sub(tmp, ht[kt][:, sl], Mt)
                            nc.vector.tensor_mul(htb[kt][:, sl], tmp, Rt)

                    wvt_t = p4.tile([128, KT, C], BF16, tag="wvt")
                    nc.sync.dma_start(out=wvt_t, in_=wvt_d[:, :, :])
                    wqt_t = p4.tile([128, KT, C], BF16, tag="wqt")
                    nc.scalar.dma_start(out=wqt_t, in_=wqt_d[:, :, :])
                    wkt_t = p4.tile([128, KT, C], BF16, tag="wkt")
                    nc.scalar.dma_start(out=wkt_t, in_=wkt_d[:, :, :])
                    wot_t = p4.tile([81, H, KT, 128], BF16, tag="wot")
                    nc.sync.dma_start(out=wot_t, in_=wot_d[:, :, :, :])

                    vt3sb = p4.tile([128, NQ // 128, H, 81], BF16,
                                    tag="vt3sb")
                    nc.vector.memset(vt3sb[:, :, :, 80], 1.0)
                    for tt in range(NQ // 128):
                        tsl = slice(tt * 128, (tt + 1) * 128)
                        for hf in range(2):
                            csl = slice(hf * 320, (hf + 1) * 320)
                            ps = ps512.tile([128, 320], F32, tag="ps")
                            for kt in range(KT):
                                nc.tensor.matmul(ps, htb[kt][:, tsl],
                                                 wvt_t[:, kt, csl],
                                                 start=(kt == 0),
                                                 stop=(kt == KT - 1))
                            nc.vector.tensor_add(
                                vt3sb[:, tt, hf * 4:(hf + 1) * 4, 0:80],
                                ps.rearrange("p (h c) -> p h c", c=DH),
                                bvtbc[:, csl].rearrange("p (h c) -> p h c",
                                                        c=DH))

                    attDt = []
                    for h in range(H):
                        hsl = slice(h * DH, (h + 1) * DH)
                        qth = p4s.tile([80, NQ], BF16, tag="qth", bufs=2)
                        kth = p4s.tile([80, NQ], BF16, tag="kth", bufs=2)
                        for ch in range(NQ // 512):
                            sl = slice(ch * 512, (ch + 1) * 512)
                            ps = ps512.tile([80, 512], F32, tag="ps")
                            for kt in range(KT):
                                nc.tensor.matmul(ps, wqt_t[:, kt, hsl],
                                                 htb[kt][:, sl],
                                                 start=(kt == 0),
                                                 stop=(kt == KT - 1))
                            nc.scalar.activation(out=qth[:, sl], in_=ps,
                                                 func=AF.Identity,
                                                 bias=bqth_t[:, h:h + 1])
                            ps2 = ps512.tile([80, 512], F32, tag="ps")
                            for kt in range(KT):
                                nc.tensor.matmul(ps2, wkt_t[:, kt, hsl],
                                                 htb[kt][:, sl],
                                                 start=(kt == 0),
                                                 stop=(kt == KT - 1))
                            nc.scalar.activation(out=kth[:, sl], in_=ps2,
                                                 func=AF.Identity,
                                                 bias=bkth_t[:, h:h + 1])
                        attP = p4w.tile([81, NQ], F32, tag="attPt", bufs=1)
                        for tt in range(NQ // 128):
                            tsl = slice(tt * 128, (tt + 1) * 128)
                            ps_s = ps512.tile([128, 128], F32, tag="ps")
                            nc.tensor.matmul(ps_s, kth[:, tsl], qth[:, tsl],
                                             start=True, stop=True)
                            Pe = p4w.tile([128, 128], BF16, tag="Pe")
                            nc.scalar.activation(out=Pe, in_=ps_s,
                                                 func=AF.Exp)
                            Pm = p4w.tile([128, 128], BF16, tag="Pm")
                            nc.vector.tensor_mul(Pm, Pe, mask_t)
                            pv = pspv.tile([81, 128], F32, tag="pvt")
                            nc.tensor.matmul(pv, vt3sb[:, tt, h, :], Pm,
                                             start=True, stop=True)
                            nc.vector.tensor_copy(attP[:, tsl], pv)
                        dnm0 = p4w.tile([1, NQ], F32, tag="dnm0t", bufs=2)
                        nc.sync.dma_start(out=dnm0, in_=attP[80:81, :])
                        nc.vector.reciprocal(out=dnm0, in_=dnm0)
                        attB = p4w.tile([80, NQ], F32, tag="attBt", bufs=1)
                        nc.gpsimd.partition_broadcast(attB, dnm0[0:1, :],
                                                      channels=80)
                        aD = p4d.tile([81, NQ], BF16, tag="attDt",
                                      name="attDt")
                        nc.vector.memset(aD, 0.0)
                        nc.vector.tensor_mul(aD[0:80, :], attP[0:80, :],
                                             attB)
                        attDt.append(aD)

                    for m in range(KT):
                        for ch in range(NQ // 512):
                            sl = slice(ch * 512, (ch + 1) * 512)
                            ps = ps512.tile([128, 512], F32, tag="ps")
                            for h in range(H):
                                nc.tensor.matmul(
                                    ps, wot_t[:, h, m, :], attDt[h][:, sl],
                                    start=(h == 0), stop=(h == H - 1))
                            oe = p4w.tile([128, 512], F32, tag="oe", bufs=2)
                            nc.vector.scalar_tensor_tensor(
                                out=oe, in0=ps,
                                scalar=bot_t[:, m:m + 1],
                                in1=ht[m][:, sl], op0=OP.add, op1=OP.add)
                            oeng = nc.scalar if ch % 2 else nc.sync
                            oeng.dma_start(out=out_d[m, :, sl], in_=oe)

            for it in range(reps):
                emit(it)

    nc.compile()
    _BUILD_CACHE[key] = nc
    return nc


def _prep_inputs(hidden_states, ln1_g, ln1_b, q1w, k1w, v1w, o1w, o1b,
                 ln3_g, ln3_b, ff_w1, ff_b1, ff_w2, ff_b2,
                 lnt_g, lnt_b, qtw, ktw, vtw, otw, otb):
    """Host-side weight folding + per-core input shards."""
    bf = ml_dtypes.bfloat16
    sc = 1.0 / math.sqrt(DH)

    def fold(g, b, w):
        return g[:, None] * w, b @ w

    wq, bq = fold(ln1_g, ln1_b, q1w)
    wq, bq = wq * sc, bq * sc
    wk, bk = fold(ln1_g, ln1_b, k1w)
    wv, bv = fold(ln1_g, ln1_b, v1w)
    # o1w quirk: channel-major interleave -> padded per-head [81, C] with the
    # original row dh*H + h at padded position (h, dh); row 80 is zero
    # (multiplies the softmax-denominator row).
    wo1 = np.zeros((H, 81, C), np.float32)
    idx_dh = np.arange(DH)
    for h in range(H):
        wo1[h, 0:DH, :] = o1w[idx_dh * H + h, :]
    wf1, bf1 = fold(ln3_g, ln3_b, ff_w1)
    bf1 = bf1 + ff_b1
    wqt, bqt = fold(lnt_g, lnt_b, qtw)
    wqt, bqt = wqt * sc, bqt * sc
    wkt, bkt = fold(lnt_g, lnt_b, ktw)
    wvt, bvt = fold(lnt_g, lnt_b, vtw)
    wot = np.zeros((H, 81, C), np.float32)
    for h in range(H):
        wot[h, 0:DH, :] = otw[h * DH + idx_dh, :]

    # 8 sequences per 128-token tile; block-diag of 8 16x16 blocks
    mask = np.kron(np.eye(8, dtype=np.float32), np.ones((F, F), np.float32))

    def c(a, dt=bf):
        return np.ascontiguousarray(np.asarray(a, np.float32).astype(dt))

    wf1 = np.asarray(wf1, np.float32)
    wf1a = wf1[:, :INNER].reshape(KT, 128, NI, 128).transpose(2, 1, 0, 3)
    wf1g = wf1[:, INNER:].reshape(KT, 128, NI, 128).transpose(2, 1, 0, 3)
    wf2h = np.asarray(ff_w2, np.float32).reshape(NI, 128, KT, 128) \
        .transpose(1, 0, 2, 3)
    woth = wot.reshape(H, 81, KT, 128).transpose(1, 0, 2, 3)

    def colmaj(b, ncol):
        return np.ascontiguousarray(
            np.asarray(b, np.float32).reshape(ncol, -1).T)

    def pkc(w):
        return c(np.asarray(w, np.float32).reshape(KT, 128, C)
                 .transpose(1, 0, 2))

    shared = dict(
        wq=pkc(wq), wk=pkc(wk), wv=pkc(wv), wo1=c(wo1),
        wf1a=c(wf1a), wf1g=c(wf1g), wf2=c(wf2h),
        wqt=pkc(wqt), wkt=pkc(wkt), wvt=pkc(wvt), wot=c(woth),
        bqh=colmaj(bq, H), bkh=colmaj(bk, H),
        bvbc=np.ascontiguousarray(
            np.broadcast_to(np.asarray(bv, np.float32), (128, C))),
        bo1=colmaj(o1b, KT), bf1c=colmaj(bf1, 2 * NI), bf2=colmaj(ff_b2, KT),
        bqth=colmaj(bqt, H), bkth=colmaj(bkt, H),
        bvtbc=np.ascontiguousarray(
            np.broadcast_to(np.asarray(bvt, np.float32), (128, C))),
        bot=colmaj(otb, KT),
        mask=c(mask),
    )

    hs = np.asarray(hidden_states, np.float32)   # [BF, D, C]
    in_maps = []
    for i in range(NCORES):
        fa, fb = 2 * i, 2 * i + 1
        fprev = max(2 * i - 1, 0)
        frames = [0, fprev, fa, fb]
        xu = hs[frames].reshape(NU, C).T          # [C, NU] feature-major
        m = dict(shared)
        m["xub"] = np.ascontiguousarray(xu.astype(bf).reshape(KT, 128, NU))
        m["xuq"] = np.ascontiguousarray(
            xu[:, NQ:].astype(np.float32).reshape(KT, 128, NQ))
        in_maps.append(m)
    return in_maps


class _Runner:
    """One shard_map jit per build variant, reused across calls.

    The stock run_bass_kernel_spmd path rebuilds the jit closure on every
    call, so each launch re-traces, re-lowers and reloads the NEFF through
    the axon tunnel (seconds).  Building it once keeps steady-state launch
    cost at one dispatch round trip, and device-resident inputs make the
    in-program reps slope an honest measure of per-iteration HW time.
    """

    def __init__(self, nc):
        self.nc = nc
        _b2j.install_neuronx_cc_hook()
        pname = nc.partition_id_tensor.name if nc.partition_id_tensor else None
        in_names, out_names, out_avals, zero_outs = [], [], [], []
        for alloc in nc.m.functions[0].allocations:
            if not isinstance(alloc, mybir.MemoryLocationSet):
                continue
            name = alloc.memorylocations[0].name
            if alloc.kind == "ExternalInput":
                if name != pname:
                    in_names.append(name)
            elif alloc.kind == "ExternalOutput":
                out_names.append(name)
                shape = tuple(alloc.tensor_shape)
                dtype = mybir.dt.np(alloc.dtype)
                out_avals.append(jax.core.ShapedArray(shape, dtype))
                zero_outs.append(np.zeros(shape, dtype))
        self.in_names = in_names[:]
        self.out_names = out_names
        self.out_avals = out_avals
        self.zero_outs = zero_outs
        n_params = len(in_names)
        bind_names = in_names + out_names + ([pname] if pname else [])

        def _body(*args):
            operands = list(args)
            if pname is not None:
                operands.append(_b2j.partition_id_tensor())
            return tuple(_b2j._bass_exec_p.bind(
                *operands, out_avals=tuple(out_avals),
                in_names=tuple(bind_names), out_names=tuple(out_names),
                lowering_input_output_aliases=(),
                sim_require_finite=True, sim_require_nnan=True, nc=nc))

        devices = jax.devices()[:NCORES]
        assert len(devices) == NCORES
        self.mesh = Mesh(np.asarray(devices), ("core",))
        nin = n_params + len(out_names)
        self.sharding = NamedSharding(self.mesh, PartitionSpec("core"))
        self.jit = jax.jit(
            _shard_map(_body, mesh=self.mesh,
                       in_specs=(PartitionSpec("core"),) * nin,
                       out_specs=(PartitionSpec("core"),) * len(out_names),
                       check_rep=False),
            keep_unused=True)

    def _concat(self, in_maps):
        cats = [np.concatenate([np.asarray(m[nm]) for m in in_maps], axis=0)
                for nm in self.in_names]
        cats += [np.zeros((NCORES * z.shape[0], *z.shape[1:]), z.dtype)
                 for z in self.zero_outs]
        return cats

    def put(self, in_maps):
        dev = jax.device_put(self._concat(in_maps),
                             [self.sharding] * (len(self.in_names)
                                                + len(self.zero_outs)))
        jax.block_until_ready(dev)
        return dev

    def exec(self, args):
        out = self.jit(*args)
        jax.block_until_ready(out)
        return out

    def run_host(self, in_maps):
        out_arrs = self.exec(self._concat(in_maps))
        res = []
        for c in range(NCORES):
            res.append({nm: np.asarray(out_arrs[i]).reshape(
                NCORES, *self.out_avals[i].shape)[c]
                for i, nm in enumerate(self.out_names)})
        return res


_RUNNER_CACHE = {}


def _get_runner(reps=1, use_cc=True, upto=4):
    key = (reps, use_cc, upto)
    if key not in _RUNNER_CACHE:
        _RUNNER_CACHE[key] = _Runner(_build(reps=reps, use_cc=use_cc, upto=upto))
    return _RUNNER_CACHE[key]


def kernel(**inputs):
    video_length = int(np.asarray(inputs.pop("video_length")))
    assert video_length == F, f"kernel hardcodes F={F}, got {video_length}"
    in_maps = _prep_inputs(**{k: np.asarray(v) for k, v in inputs.items()})
    for attempt in range(3):
        try:
            results = _get_runner(reps=1).run_host(in_maps)
            break
        except Exception:
            # transient NRT exec-unit failures have been observed to clear on
            # the next launch; rebuild the jit and retry
            if attempt == 2:
                raise
            _RUNNER_CACHE.clear()
            jax.clear_caches()
    out = np.empty((F, D, C), np.float32)
    for i in range(NCORES):
        r = results[i]["out"].reshape(C, D // NCORES, F)   # [c, s, f]
        out[:, i * (D // NCORES):(i + 1) * (D // NCORES), :] = r.transpose(2, 1, 0)
    return out
